# revision 6
# baseline (speedup 1.0000x reference)
"""Multi-head self-attention (SAGAN-style, 1x1-conv projections) on 8 Trainium2 cores.

Problem: x [2, 256, 64, 64], 8 heads, cph=32, L=4096 tokens per batch element.
  q/k/v = 1x1 conv projections of x; att = softmax_j(k_i . q_j); out_i = sum_j att_ij v_j;
  y = gamma * (Wl @ out + bl) + x

Sharding: output-token split — core c owns (n = c//4, token chunk c%4 of 1024).
Each core needs: full Q and V for its n, K only for its chunk. No collectives.

Per-core kernel (all matmuls bf16 with fp32 PSUM accumulation):
  S^T[j, i] = sum_c q'[c,j] k[c,i]   (q' pre-scaled by cph^-0.5; softmax over j)
    -> 4 heads row-packed in the 128x128 PE array (K=32 each, tile_position=(32h,0))
  P^T = exp(S^T)  on ACT, PSUM -> SBUF bf16 (scores bounded ~0.8, no max-subtraction
    needed). This is the bottleneck: 33.5M exps/core at 1 elem/cycle/lane ~ 255us busy.
  AV[d, i] = sum_j v^T[j,d] P^T[j,i]  -> 4 heads col-packed (M=32, tile_position=(0,32h))
  denom[i] = sum_j P^T[j,i]           -> ones-vector matmuls, M=1, col-packed
  attn = AV * (1/denom broadcast via stream_shuffle)
  y = Wl_gamma @ attn + bl_gamma + x_residual   (gamma folded into Wl/bl host-side)

PSUM budget (8 banks): 3-slot ring of [128,1024] tiles (6 banks) shared by S^T/exp
staging and all projection psum, + AV/denominator accumulators (2 banks).

Emission order streams the projections: the ramp holds only the minimal first-exp
chain; the rest (V^T in 4-j chunks, Q' in 2-slice pairs, second K slice) is emitted
inside the first section's j-loop, spread so at most one [128,1024] psum injection
lands per j-step. Input DMAs are issued critical-path-first, round-robin across
three DMA queues (sync/scalar/gpsimd) to beat the ~1us-per-DMA dispatch serial cost.
"""
import numpy as np
import ml_dtypes

bf16 = ml_dtypes.bfloat16

N, C, H, W = 2, 256, 64, 64
L = H * W            # 4096
NH = 8               # heads
CPH = C // NH        # 32
NCORES = 8
CHUNK = 1024         # output tokens per core (L / 4)
P = 128

_cache = {}


def _build():
    import concourse.mybir as mybir
    import concourse.tile as tile
    from concourse import bacc

    FP32 = mybir.dt.float32
    BF16 = mybir.dt.bfloat16
    EXP = mybir.ActivationFunctionType.Exp
    ADD = mybir.AluOpType.add

    nc = bacc.Bacc("TRN2", target_bir_lowering=False, debug=False,
                   num_devices=NCORES)

    xb_d = nc.dram_tensor("xb", [C, L], BF16, kind="ExternalInput").ap()
    xk_d = nc.dram_tensor("xk", [C, CHUNK], BF16, kind="ExternalInput").ap()
    xres_d = nc.dram_tensor("xres", [C, CHUNK], FP32, kind="ExternalInput").ap()
    wq_d = nc.dram_tensor("wqt", [C, C], BF16, kind="ExternalInput").ap()
    wk_d = nc.dram_tensor("wkt", [C, C], BF16, kind="ExternalInput").ap()
    wv_d = nc.dram_tensor("wvt", [C, C], BF16, kind="ExternalInput").ap()
    wl_d = nc.dram_tensor("wlt", [C, C], BF16, kind="ExternalInput").ap()
    bq_d = nc.dram_tensor("bq2", [C, 1], FP32, kind="ExternalInput").ap()
    bk_d = nc.dram_tensor("bk2", [C, 1], FP32, kind="ExternalInput").ap()
    bl_d = nc.dram_tensor("bl2", [C, 1], FP32, kind="ExternalInput").ap()
    bv_d = nc.dram_tensor("bv2", [1, C], BF16, kind="ExternalInput").ap()
    y_d = nc.dram_tensor("y", [C, CHUNK], FP32, kind="ExternalOutput").ap()

    JT = L // P          # 32 j-tiles
    TS = L // 512        # 8 token slices of x
    NP = TS // 2         # 4 Q' slice-pairs per head-group

    with tile.TileContext(nc) as tc:
        with tc.tile_pool(name="consts", bufs=1) as consts, \
             tc.tile_pool(name="data", bufs=1) as data, \
             tc.tile_pool(name="ptp", bufs=6) as ptp, \
             tc.tile_pool(name="ep", bufs=2) as ep, \
             tc.tile_pool(name="attnp", bufs=2) as attnp, \
             tc.tile_pool(name="ring", bufs=3, space="PSUM") as ringp, \
             tc.tile_pool(name="accp", bufs=1, space="PSUM") as accp:

            # round-robin DMA queues; emission order = priority order, so
            # everything below is issued most-critical-first
            _queues = [nc.sync, nc.scalar, nc.gpsimd]
            _qi = [0]

            def dma(out, in_):
                eng = _queues[_qi[0] % 3]
                _qi[0] += 1
                eng.dma_start(out=out, in_=in_)

            # ---- tile declarations (no DMAs yet)
            def wtile(nm, ct):
                return consts.tile([P, C], BF16, name=f"{nm}{ct}")

            def btile(nm, ct):
                return consts.tile([P, 1], FP32, name=f"{nm}{ct}")

            wk_sb = [wtile("wk", ct) for ct in range(2)]
            wq_sb = [wtile("wq", ct) for ct in range(2)]
            wv_sb = [wtile("wv", ct) for ct in range(2)]
            bk_sb = [btile("bk", ct) for ct in range(2)]
            bq_sb = [btile("bq", ct) for ct in range(2)]
            bv_sb = consts.tile([1, C], BF16, name="bv")
            xk_sb = [data.tile([P, CHUNK], BF16, name=f"xk{ct}")
                     for ct in range(2)]
            xb_sb = [[data.tile([P, 512], BF16, name=f"xb{ct}_{t2}")
                      for t2 in range(TS)] for ct in range(2)]

            ones_col = consts.tile([P, 1], BF16, name="ones_col")
            ones_row = consts.tile([1, P], BF16, name="ones_row")
            nc.vector.memset(ones_col, 1.0)
            nc.vector.memset(ones_row, 1.0)

            # critical-path-first loads
            for ct in range(2):
                dma(wk_sb[ct], wk_d[ct * P:(ct + 1) * P, :])
            for ct in range(2):
                dma(xk_sb[ct], xk_d[ct * P:(ct + 1) * P, :])
            for ct in range(2):
                dma(wq_sb[ct], wq_d[ct * P:(ct + 1) * P, :])
            for ct in range(2):
                dma(xb_sb[ct][0], xb_d[ct * P:(ct + 1) * P, 0:512])
            for ct in range(2):
                dma(bk_sb[ct], bk_d[ct * P:(ct + 1) * P, :])
                dma(bq_sb[ct], bq_d[ct * P:(ct + 1) * P, :])
            for ct in range(2):
                dma(wv_sb[ct], wv_d[ct * P:(ct + 1) * P, :])
            dma(bv_sb, bv_d)
            for t2 in range(1, TS):
                for ct in range(2):
                    dma(xb_sb[ct][t2], xb_d[ct * P:(ct + 1) * P,
                                            t2 * 512:(t2 + 1) * 512])

            # ---- projection outputs
            # Q' split in [128,1024] pair-tiles so consumers only dep on their slice
            qp_sb = [[data.tile([P, 1024], BF16, name=f"qp{hg}_{pr}")
                      for pr in range(NP)] for hg in range(2)]
            kk_sb = [data.tile([P, CHUNK], BF16, name=f"kk{hg}")
                     for hg in range(2)]
            vt_sb = data.tile([P, JT * C], BF16, name="vt")  # col j*256+h*32+d

            def q_ap(hg, j):
                return qp_sb[hg][j // 8][:, (j % 8) * P:(j % 8 + 1) * P]

            # ---- projection emitters (psum from the shared ring pool,
            # always a full [128,1024] slot)
            def emit_kproj(hg):
                ps = ringp.tile([P, 1024], FP32, name="ring")
                for ih in range(2):
                    for ct in range(2):
                        nc.tensor.matmul(
                            out=ps[:, ih * 512:(ih + 1) * 512],
                            lhsT=wk_sb[ct][:, hg * P:(hg + 1) * P],
                            rhs=xk_sb[ct][:, ih * 512:(ih + 1) * 512],
                            start=(ct == 0), stop=(ct == 1))
                nc.vector.tensor_scalar(out=kk_sb[hg], in0=ps,
                                        scalar1=bk_sb[hg], scalar2=None, op0=ADD)

            def emit_qproj(hg, pr):
                ps = ringp.tile([P, 1024], FP32, name="ring")
                for k in range(2):
                    for ct in range(2):
                        nc.tensor.matmul(
                            out=ps[:, k * 512:(k + 1) * 512],
                            lhsT=wq_sb[ct][:, hg * P:(hg + 1) * P],
                            rhs=xb_sb[ct][2 * pr + k],
                            start=(ct == 0), stop=(ct == 1))
                nc.vector.tensor_scalar(out=qp_sb[hg][pr], in0=ps,
                                        scalar1=bq_sb[hg], scalar2=None, op0=ADD)

            def emit_vchunk(c):
                # V^T for j = 4c..4c+3 in one [128,1024] ring slot
                ps = ringp.tile([P, 1024], FP32, name="ring")
                for jj in range(4):
                    j = 4 * c + jj
                    sub = ps[:, jj * C:(jj + 1) * C]
                    for ct in range(2):
                        nc.tensor.matmul(
                            out=sub,
                            lhsT=xb_sb[ct][j // 4][:, (j % 4) * P:(j % 4 + 1) * P],
                            rhs=wv_sb[ct], start=(ct == 0), stop=False)
                    nc.tensor.matmul(out=sub, lhsT=ones_row, rhs=bv_sb,
                                     start=False, stop=True)
                nc.vector.tensor_copy(
                    vt_sb[:, c * 4 * C:(c + 1) * 4 * C], ps)

            # ---- attention section: one (ihalf, head-group) pass over all j
            def emit_section(ihalf, hg, extra=None):
                acc_av = accp.tile([P, 512], FP32, name="acc_av")
                acc_dn = accp.tile([P, 512], FP32, name="acc_dn")
                for j in range(JT):
                    rA = ringp.tile([P, 1024], FP32, name="ring")
                    rB = ringp.tile([P, 1024], FP32, name="ring")
                    for hh in range(4):
                        rt = rA if hh < 2 else rB
                        col = (hh % 2) * 512
                        nc.tensor.matmul(
                            out=rt[:, col:col + 512],
                            lhsT=q_ap(hg, j)[32 * hh:32 * hh + 32, :],
                            rhs=kk_sb[hg][32 * hh:32 * hh + 32,
                                          ihalf * 512:(ihalf + 1) * 512],
                            start=True, stop=True,
                            tile_position=(32 * hh, 0))
                    ptA = ptp.tile([P, 1024], BF16, name="pt")
                    ptB = ptp.tile([P, 1024], BF16, name="pt")
                    nc.scalar.activation(out=ptA, in_=rA, func=EXP)
                    nc.scalar.activation(out=ptB, in_=rB, func=EXP)
                    for hh in range(4):
                        pt = ptA if hh < 2 else ptB
                        col = (hh % 2) * 512
                        h = hg * 4 + hh
                        nc.tensor.matmul(
                            out=acc_av[32 * hh:32 * hh + 32, :],
                            lhsT=vt_sb[:, j * C + 32 * h:j * C + 32 * h + 32],
                            rhs=pt[:, col:col + 512],
                            start=(j == 0), stop=(j == JT - 1),
                            tile_position=(0, 32 * hh))
                    for hh in range(4):
                        pt = ptA if hh < 2 else ptB
                        col = (hh % 2) * 512
                        nc.tensor.matmul(
                            out=acc_dn[32 * hh:32 * hh + 1, :],
                            lhsT=ones_col,
                            rhs=pt[:, col:col + 512],
                            start=(j == 0), stop=(j == JT - 1),
                            tile_position=(0, 32 * hh))
                    if extra is not None:
                        extra(j)
                # normalize: attn = AV / denom
                db = ep.tile([P, 512], FP32, name="db")
                for hh in range(4):
                    nc.vector.stream_shuffle(
                        out=db[32 * hh:32 * hh + 32, :],
                        in_=acc_dn[32 * hh:32 * hh + 32, :],
                        mask=[0] * 32)
                rcp = ep.tile([P, 512], FP32, name="rcp")
                nc.vector.reciprocal(rcp, db)
                attn = attnp.tile([P, 512], BF16, name=f"attn{hg}")
                nc.vector.tensor_mul(attn, acc_av, rcp)
                return attn

            def emit_outproj(ihalf, attn_sb):
                pso = ringp.tile([P, 1024], FP32, name="ring")
                for ot in range(2):
                    sub = pso[:, ot * 512:(ot + 1) * 512]
                    for ct in range(2):
                        nc.tensor.matmul(
                            out=sub, lhsT=wl_sb[ct][:, ot * P:(ot + 1) * P],
                            rhs=attn_sb[ct], start=(ct == 0), stop=(ct == 1))
                for ot in range(2):
                    sub = pso[:, ot * 512:(ot + 1) * 512]
                    t1 = ep.tile([P, 512], FP32, name="t1")
                    nc.vector.tensor_scalar(out=t1, in0=sub, scalar1=bl_sb[ot],
                                            scalar2=None, op0=ADD)
                    yt = ep.tile([P, 512], FP32, name="yt")
                    nc.vector.tensor_add(
                        yt, t1, xres_sb[ot][:, ihalf * 512:(ihalf + 1) * 512])
                    nc.sync.dma_start(
                        out=y_d[ot * P:(ot + 1) * P,
                                ihalf * 512:(ihalf + 1) * 512],
                        in_=yt)

            # ---- schedule
            # ramp: minimal chain to the first exp + V^T for j<8
            emit_kproj(0)
            emit_qproj(0, 0)
            emit_vchunk(0)
            emit_vchunk(1)

            wl_sb = []
            bl_sb = []
            xres_sb = []

            def extra00(j):
                if j == 0:
                    emit_kproj(1)
                    emit_vchunk(2)                # vt j 8..11  (lead 8 steps)
                if j >= 4 and j % 4 == 0 and j // 4 + 2 < TS:
                    emit_vchunk(j // 4 + 2)       # vt j+8..j+11 (lead 8 steps)
                if j in (2, 10, 18):
                    emit_qproj(0, j // 8 + 1)     # q'hg0 pair (lead >=6 steps)
                if j in (13, 17, 21, 25):
                    emit_qproj(1, (j - 13) // 4)  # q'hg1 for section (0,1)
                if j == 9:
                    # deferred loads only needed by the output projection
                    for ct in range(2):
                        t = consts.tile([P, C], BF16, name=f"wl{ct}")
                        dma(t, wl_d[ct * P:(ct + 1) * P, :])
                        wl_sb.append(t)
                        t = consts.tile([P, 1], FP32, name=f"bl{ct}")
                        dma(t, bl_d[ct * P:(ct + 1) * P, :])
                        bl_sb.append(t)
                        t = data.tile([P, CHUNK], FP32, name=f"xres{ct}")
                        dma(t, xres_d[ct * P:(ct + 1) * P, :])
                        xres_sb.append(t)

            a00 = emit_section(0, 0, extra=extra00)
            a01 = emit_section(0, 1)
            emit_outproj(0, [a00, a01])
            a10 = emit_section(1, 0)
            a11 = emit_section(1, 1)
            emit_outproj(1, [a10, a11])

    nc.compile()
    return nc


def _get_nc():
    if "nc" not in _cache:
        _cache["nc"] = _build()
    return _cache["nc"]


def _prep_in_maps(x, wq, bq, wk, bk, wv, bv, wl, bl, gamma):
    x = np.asarray(x, dtype=np.float32)
    scale = float(CPH) ** -0.5
    g = float(np.asarray(gamma).reshape(-1)[0])

    wqsT = np.ascontiguousarray((np.asarray(wq, np.float32) * scale).T).astype(bf16)
    wkT = np.ascontiguousarray(np.asarray(wk, np.float32).T).astype(bf16)
    wvT = np.ascontiguousarray(np.asarray(wv, np.float32).T).astype(bf16)
    wlgT = np.ascontiguousarray((np.asarray(wl, np.float32) * g).T).astype(bf16)
    bq2 = (np.asarray(bq, np.float32) * scale).reshape(C, 1)
    bk2 = np.asarray(bk, np.float32).reshape(C, 1)
    bl2 = (np.asarray(bl, np.float32) * g).reshape(C, 1)
    bv2 = np.asarray(bv, np.float32).astype(bf16).reshape(1, C)

    xf = x.reshape(N, C, L)
    xbs = [xf[n].astype(bf16) for n in range(N)]

    in_maps = []
    for c in range(NCORES):
        n, ch = c // 4, c % 4
        isl = slice(ch * CHUNK, (ch + 1) * CHUNK)
        in_maps.append({
            "xb": xbs[n],
            "xk": np.ascontiguousarray(xbs[n][:, isl]),
            "xres": np.ascontiguousarray(xf[n][:, isl]),
            "wqt": wqsT, "wkt": wkT, "wvt": wvT, "wlt": wlgT,
            "bq2": bq2, "bk2": bk2, "bl2": bl2, "bv2": bv2,
        })
    return in_maps


def kernel(x=None, wq=None, bq=None, wk=None, bk=None, wv=None, bv=None,
           wl=None, bl=None, gamma=None, num_heads=None, **_unused):
    from concourse import bass_utils

    x = np.asarray(x, dtype=np.float32)
    assert x.shape == (N, C, H, W), f"unexpected x shape {x.shape}"
    assert int(np.asarray(num_heads)) == NH

    in_maps = _prep_in_maps(x, wq, bq, wk, bk, wv, bv, wl, bl, gamma)
    nc = _get_nc()
    res = bass_utils.run_bass_kernel_spmd(nc, in_maps,
                                          core_ids=list(range(NCORES)))

    out = np.empty((N, C, L), np.float32)
    for c in range(NCORES):
        n, ch = c // 4, c % 4
        out[n][:, ch * CHUNK:(ch + 1) * CHUNK] = res.results[c]["y"]
    return out.reshape(N, C, H, W)


if __name__ == "__main__":
    import reference
    inputs = reference.setup_inputs()
    expected = np.asarray(reference.reference(**inputs))
    got = kernel(**{k: np.asarray(v) if hasattr(v, "shape") else v
                    for k, v in inputs.items()})
    rel = np.linalg.norm(got - expected) / np.linalg.norm(expected)
    print("rel err:", rel)


# revision 18
# speedup vs baseline: 35.6294x; 35.6294x over previous
"""Multi-head self-attention (SAGAN-style, 1x1-conv projections) on 8 Trainium2 cores.

Problem: x [2, 256, 64, 64], 8 heads, cph=32, L=4096 tokens per batch element.
  q/k/v = 1x1 conv projections of x; att = softmax_j(k_i . q_j); out_i = sum_j att_ij v_j;
  y = gamma * (Wl @ out + bl) + x

Sharding: output-token split — core c owns (n = c//4, token chunk c%4 of 1024).
Each core needs: full Q and V for its n, K only for its chunk. No collectives.

Per-core kernel (all matmuls bf16 with fp32 PSUM accumulation):
  S^T[j, i] = sum_c q'[c,j] k[c,i]   (q' pre-scaled by cph^-0.5; softmax over j)
    -> 4 heads row-packed in the 128x128 PE array (K=32 each, tile_position=(32h,0))
  P^T = exp(S^T)  on ACT, PSUM -> SBUF bf16 (scores bounded ~0.8, no max-subtraction
    needed). This is the bottleneck: 33.5M exps/core at 1 elem/cycle/lane ~ 255us busy.
  AV[d, i] = sum_j v^T[j,d] P^T[j,i]  -> 4 heads col-packed (M=32, tile_position=(0,32h))
  denom[i] = sum_j P^T[j,i]           -> ones-matrix matmuls (M=32 so every row of
    the head block carries the sum - no broadcast needed), col-packed
  attn = AV * reciprocal(denom)
  y = Wl_gamma @ attn + bl_gamma + x_residual   (gamma folded into Wl/bl host-side)

PSUM budget (8 banks): 3-slot ring of [128,1024] tiles (6 banks) shared by S^T/exp
staging and all projection psum, + AV/denominator accumulators (2 banks).

Emission order streams the projections: the ramp holds only the minimal first-exp
chain; the rest (V^T in 4-j chunks, Q' in 2-slice pairs, second K slice) is emitted
inside the first section's j-loop, spread so at most one [128,1024] psum injection
lands per j-step. Input DMAs are issued critical-path-first, round-robin across
three DMA queues (sync/scalar/gpsimd) to beat the ~1us-per-DMA dispatch serial cost.
"""
import numpy as np
import ml_dtypes

bf16 = ml_dtypes.bfloat16

N, C, H, W = 2, 256, 64, 64
L = H * W            # 4096
NH = 8               # heads
CPH = C // NH        # 32
NCORES = 8
CHUNK = 1024         # output tokens per core (L / 4)
P = 128

_cache = {}


def _build():
    import concourse.mybir as mybir
    import concourse.tile as tile
    from concourse import bacc

    FP32 = mybir.dt.float32
    BF16 = mybir.dt.bfloat16
    EXP = mybir.ActivationFunctionType.Exp
    CPY = mybir.ActivationFunctionType.Copy
    ADD = mybir.AluOpType.add

    nc = bacc.Bacc("TRN2", target_bir_lowering=False, debug=False,
                   num_devices=NCORES)

    xb_d = nc.dram_tensor("xb", [C, L], BF16, kind="ExternalInput").ap()
    xres_d = nc.dram_tensor("xres", [C, CHUNK], FP32, kind="ExternalInput").ap()
    # wx = [wkT | wqT | wvT | wlT | xk] packed; ball = [bk | bq | bl | 0]
    wx_d = nc.dram_tensor("wx", [C, 4 * C + CHUNK], BF16,
                          kind="ExternalInput").ap()
    ball_d = nc.dram_tensor("ball", [C, 4], FP32, kind="ExternalInput").ap()
    # brows = [bv | bk | bq'] as rows, for K=1 bias-fold matmuls
    brows_d = nc.dram_tensor("brows", [1, 3 * C], BF16, kind="ExternalInput").ap()
    y_d = nc.dram_tensor("y", [C, CHUNK], FP32, kind="ExternalOutput").ap()

    JT = L // P          # 32 j-tiles
    TS = L // 512        # 8 token slices of x
    NP = TS // 2         # 4 Q' slice-pairs per head-group

    with tile.TileContext(nc) as tc:
        with tc.tile_pool(name="consts", bufs=1) as consts, \
             tc.tile_pool(name="data", bufs=1) as data, \
             tc.tile_pool(name="ptp", bufs=6) as ptp, \
             tc.tile_pool(name="ep", bufs=2) as ep, \
             tc.tile_pool(name="attnp", bufs=2) as attnp, \
             tc.tile_pool(name="ring", bufs=3, space="PSUM") as ringp, \
             tc.tile_pool(name="accp", bufs=1, space="PSUM") as accp:

            # round-robin DMA queues; emission order = priority order, so
            # everything below is issued most-critical-first
            _queues = [nc.sync, nc.gpsimd]
            _qi = [0]

            def dma(out, in_):
                eng = _queues[_qi[0] % 2]
                _qi[0] += 1
                eng.dma_start(out=out, in_=in_)

            # ---- tile declarations; packed weight/bias tiles
            wx_sb = [consts.tile([P, 4 * C + CHUNK], BF16, name=f"wx{ct}")
                     for ct in range(2)]
            ball_sb = [consts.tile([P, 4], FP32, name=f"ball{ct}")
                       for ct in range(2)]
            wk_sb = [w[:, 0:C] for w in wx_sb]
            wq_sb = [w[:, C:2 * C] for w in wx_sb]
            wv_sb = [w[:, 2 * C:3 * C] for w in wx_sb]
            wl_sb = [w[:, 3 * C:4 * C] for w in wx_sb]
            xk_sb = [w[:, 4 * C:4 * C + CHUNK] for w in wx_sb]
            bk_sb = [b[:, 0:1] for b in ball_sb]
            bq_sb = [b[:, 1:2] for b in ball_sb]
            bl_sb = [b[:, 2:3] for b in ball_sb]
            brows_sb = consts.tile([1, 3 * C], BF16, name="brows")
            bv_sb = brows_sb[:, 0:C]
            bkrow_sb = brows_sb[:, C:2 * C]
            bqrow_sb = brows_sb[:, 2 * C:3 * C]
            # xb per c-tile: [1024 | 1024 | 2048] pieces — the first piece is
            # small so the ramp's Q'(hg0, pair0) starts early; later pieces are
            # big to amortize the ~1.7us fixed per-DMA dispatch cost
            xbh_sb = [[data.tile([P, sz], BF16, name=f"xb{ct}_{pi}")
                       for pi, sz in enumerate((1024, 1024, 2048))]
                      for ct in range(2)]
            _piece = {0: (0, 0), 1: (0, 512), 2: (1, 0), 3: (1, 512),
                      4: (2, 0), 5: (2, 512), 6: (2, 1024), 7: (2, 1536)}

            def xb_ap(ct, t2, off=0, width=512):
                pi, base = _piece[t2]
                return xbh_sb[ct][pi][:, base + off:base + off + width]

            ones_c32 = consts.tile([P, 32], BF16, name="ones_c32")
            ones_big = consts.tile([1, 512], BF16, name="ones_big")
            ones_row = ones_big[:, 0:P]
            nc.vector.memset(ones_c32, 1.0)
            nc.vector.memset(ones_big, 1.0)

            # critical-path-first loads with explicit queue assignment.
            # The scalar(ACT) queue is used for exactly two ramp DMAs that
            # finish before the first exp is ready; everything later stays off
            # the ACT queue.
            nc.sync.dma_start(out=wx_sb[0], in_=wx_d[0:P, :])
            nc.scalar.dma_start(out=wx_sb[1], in_=wx_d[P:2 * P, :])
            nc.gpsimd.dma_start(out=xbh_sb[0][0], in_=xb_d[0:P, 0:1024])
            nc.gpsimd.dma_start(out=xbh_sb[1][0], in_=xb_d[P:2 * P, 0:1024])
            nc.sync.dma_start(out=brows_sb, in_=brows_d)
            nc.sync.dma_start(out=ball_sb[0], in_=ball_d[0:P, :])
            nc.sync.dma_start(out=ball_sb[1], in_=ball_d[P:2 * P, :])
            nc.gpsimd.dma_start(out=xbh_sb[0][1], in_=xb_d[0:P, 1024:2048])
            nc.sync.dma_start(out=xbh_sb[1][1], in_=xb_d[P:2 * P, 1024:2048])
            nc.gpsimd.dma_start(out=xbh_sb[0][2], in_=xb_d[0:P, 2048:4096])
            nc.sync.dma_start(out=xbh_sb[1][2], in_=xb_d[P:2 * P, 2048:4096])

            # ---- projection outputs
            # Q' split in [128,1024] pair-tiles so consumers only dep on their slice
            qp_sb = [[data.tile([P, 1024], BF16, name=f"qp{hg}_{pr}")
                      for pr in range(NP)] for hg in range(2)]
            kk_sb = [data.tile([P, CHUNK], BF16, name=f"kk{hg}")
                     for hg in range(2)]
            vt_sb = data.tile([P, JT * C], BF16, name="vt")  # col j*256+h*32+d

            def q_ap(hg, j):
                return qp_sb[hg][j // 8][:, (j % 8) * P:(j % 8 + 1) * P]

            # ---- projection emitters (psum from the shared ring pool,
            # always a full [128,1024] slot)
            def emit_kproj(hg, use_act=False):
                ps = ringp.tile([P, 1024], FP32, name="ring")
                for ih in range(2):
                    for ct in range(2):
                        nc.tensor.matmul(
                            out=ps[:, ih * 512:(ih + 1) * 512],
                            lhsT=wk_sb[ct][:, hg * P:(hg + 1) * P],
                            rhs=xk_sb[ct][:, ih * 512:(ih + 1) * 512],
                            start=(ct == 0), stop=(ct == 1 and not use_act),
                            skip_group_check=use_act)
                if use_act:
                    # bias folded in by emit-time K=1 matmuls; ACT does the
                    # pure copy (ACT is idle during the ramp; keeps DVE free)
                    for ih in range(2):
                        nc.tensor.matmul(
                            out=ps[:, ih * 512:(ih + 1) * 512],
                            lhsT=bkrow_sb[:, hg * P:(hg + 1) * P],
                            rhs=ones_big, start=False, stop=True,
                            skip_group_check=True)
                    nc.scalar.activation(out=kk_sb[hg], in_=ps, func=CPY)
                else:
                    nc.vector.tensor_scalar(out=kk_sb[hg], in0=ps,
                                            scalar1=bk_sb[hg], scalar2=None,
                                            op0=ADD)

            def emit_qproj(hg, pr, use_act=False):
                ps = ringp.tile([P, 1024], FP32, name="ring")
                for k in range(2):
                    for ct in range(2):
                        nc.tensor.matmul(
                            out=ps[:, k * 512:(k + 1) * 512],
                            lhsT=wq_sb[ct][:, hg * P:(hg + 1) * P],
                            rhs=xb_ap(ct, 2 * pr + k),
                            start=(ct == 0), stop=(ct == 1 and not use_act),
                            skip_group_check=use_act)
                if use_act:
                    for k in range(2):
                        nc.tensor.matmul(
                            out=ps[:, k * 512:(k + 1) * 512],
                            lhsT=bqrow_sb[:, hg * P:(hg + 1) * P],
                            rhs=ones_big, start=False, stop=True,
                            skip_group_check=True)
                    nc.scalar.activation(out=qp_sb[hg][pr], in_=ps, func=CPY)
                else:
                    nc.vector.tensor_scalar(out=qp_sb[hg][pr], in0=ps,
                                            scalar1=bq_sb[hg], scalar2=None,
                                            op0=ADD)

            def emit_vchunk(c):
                # V^T for j = 4c..4c+3 in one [128,1024] ring slot
                ps = ringp.tile([P, 1024], FP32, name="ring")
                for jj in range(4):
                    j = 4 * c + jj
                    sub = ps[:, jj * C:(jj + 1) * C]
                    for ct in range(2):
                        nc.tensor.matmul(
                            out=sub,
                            lhsT=xb_ap(ct, j // 4, off=(j % 4) * P, width=P),
                            rhs=wv_sb[ct], start=(ct == 0), stop=False)
                    nc.tensor.matmul(out=sub, lhsT=ones_row, rhs=bv_sb,
                                     start=False, stop=True)
                nc.vector.tensor_copy(
                    vt_sb[:, c * 4 * C:(c + 1) * 4 * C], ps)

            # ---- attention section: one (ihalf, head-group) pass over all j
            def emit_section(ihalf, hg, extra=None):
                acc_av = accp.tile([P, 512], FP32, name="acc_av")
                acc_dn = accp.tile([P, 512], FP32, name="acc_dn")
                for j in range(JT):
                    rA = ringp.tile([P, 1024], FP32, name="ring")
                    rB = ringp.tile([P, 1024], FP32, name="ring")
                    for hh in range(4):
                        rt = rA if hh < 2 else rB
                        col = (hh % 2) * 512
                        nc.tensor.matmul(
                            out=rt[:, col:col + 512],
                            lhsT=q_ap(hg, j)[32 * hh:32 * hh + 32, :],
                            rhs=kk_sb[hg][32 * hh:32 * hh + 32,
                                          ihalf * 512:(ihalf + 1) * 512],
                            start=True, stop=True,
                            tile_position=(32 * hh, 0))
                    ptA = ptp.tile([P, 1024], BF16, name="pt")
                    ptB = ptp.tile([P, 1024], BF16, name="pt")
                    nc.scalar.activation(out=ptA, in_=rA, func=EXP)
                    nc.scalar.activation(out=ptB, in_=rB, func=EXP)
                    for hh in range(4):
                        pt = ptA if hh < 2 else ptB
                        col = (hh % 2) * 512
                        h = hg * 4 + hh
                        nc.tensor.matmul(
                            out=acc_av[32 * hh:32 * hh + 32, :],
                            lhsT=vt_sb[:, j * C + 32 * h:j * C + 32 * h + 32],
                            rhs=pt[:, col:col + 512],
                            start=(j == 0), stop=(j == JT - 1),
                            tile_position=(0, 32 * hh),
                            skip_group_check=True)
                    for hh in range(4):
                        pt = ptA if hh < 2 else ptB
                        col = (hh % 2) * 512
                        nc.tensor.matmul(
                            out=acc_dn[32 * hh:32 * hh + 32, :],
                            lhsT=ones_c32,
                            rhs=pt[:, col:col + 512],
                            start=(j == 0), stop=(j == JT - 1),
                            tile_position=(0, 32 * hh),
                            skip_group_check=True)
                    if extra is not None:
                        extra(j)
                # normalize: attn = AV / denom (the M=32 ones-matmul wrote the
                # denominator to every row of each head block, so no broadcast
                # step is needed)
                rcp = ep.tile([P, 512], FP32, name="rcp")
                nc.vector.reciprocal(rcp, acc_dn)
                attn = attnp.tile([P, 512], BF16, name=f"attn{hg}")
                nc.vector.tensor_mul(attn, acc_av, rcp)
                return attn

            def emit_outproj(ihalf, attn_sb):
                pso = ringp.tile([P, 1024], FP32, name="ring")
                for ot in range(2):
                    sub = pso[:, ot * 512:(ot + 1) * 512]
                    for ct in range(2):
                        nc.tensor.matmul(
                            out=sub, lhsT=wl_sb[ct][:, ot * P:(ot + 1) * P],
                            rhs=attn_sb[ct], start=(ct == 0), stop=(ct == 1))
                for ot in range(2):
                    sub = pso[:, ot * 512:(ot + 1) * 512]
                    t1 = ep.tile([P, 512], FP32, name="t1")
                    nc.vector.tensor_scalar(out=t1, in0=sub, scalar1=bl_sb[ot],
                                            scalar2=None, op0=ADD)
                    yt = ep.tile([P, 512], FP32, name="yt")
                    nc.vector.tensor_add(
                        yt, t1, xres_sb[ot][:, ihalf * 512:(ihalf + 1) * 512])
                    nc.sync.dma_start(
                        out=y_d[ot * P:(ot + 1) * P,
                                ihalf * 512:(ihalf + 1) * 512],
                        in_=yt)

            # ---- schedule
            # ramp: minimal chain to the first exp + V^T for j<8
            emit_kproj(0, use_act=True)
            emit_qproj(0, 0, use_act=True)
            emit_vchunk(0)

            xres_sb = []

            def extra00(j):
                if j == 0:
                    emit_vchunk(1)                # vt j 4..7   (lead 4 steps)
                if j == 1:
                    emit_kproj(1)
                if j == 2:
                    emit_vchunk(2)                # vt j 8..11  (lead 6 steps)
                if j >= 4 and j % 4 == 0 and j // 4 + 2 < TS:
                    emit_vchunk(j // 4 + 2)       # vt j+8..j+11 (lead 8 steps)
                if j in (2, 10, 18):
                    emit_qproj(0, j // 8 + 1)     # q'hg0 pair (lead >=6 steps)
                if j in (13, 17, 21, 25):
                    emit_qproj(1, (j - 13) // 4)  # q'hg1 for section (0,1)
                if j == 9:
                    # deferred load only needed by the output projection
                    for ct in range(2):
                        t = data.tile([P, CHUNK], FP32, name=f"xres{ct}")
                        dma(t, xres_d[ct * P:(ct + 1) * P, :])
                        xres_sb.append(t)

            a00 = emit_section(0, 0, extra=extra00)
            a01 = emit_section(0, 1)
            emit_outproj(0, [a00, a01])
            a10 = emit_section(1, 0)
            a11 = emit_section(1, 1)
            emit_outproj(1, [a10, a11])

    nc.compile()
    return nc


def _get_nc():
    if "nc" not in _cache:
        _cache["nc"] = _build()
    return _cache["nc"]


def _prep_in_maps(x, wq, bq, wk, bk, wv, bv, wl, bl, gamma):
    x = np.asarray(x, dtype=np.float32)
    scale = float(CPH) ** -0.5
    g = float(np.asarray(gamma).reshape(-1)[0])

    wqsT = (np.asarray(wq, np.float32) * scale).T
    wkT = np.asarray(wk, np.float32).T
    wvT = np.asarray(wv, np.float32).T
    wlgT = (np.asarray(wl, np.float32) * g).T
    wall = np.concatenate([wkT, wqsT, wvT, wlgT], axis=1).astype(bf16)
    ball = np.ascontiguousarray(np.stack(
        [np.asarray(bk, np.float32),
         np.asarray(bq, np.float32) * scale,
         np.asarray(bl, np.float32) * g,
         np.zeros(C, np.float32)], axis=1))
    bvf = np.asarray(bv, np.float32)
    brows = np.concatenate(
        [bvf, np.asarray(bk, np.float32),
         np.asarray(bq, np.float32) * scale]).astype(bf16).reshape(1, 3 * C)

    xf = x.reshape(N, C, L)
    xbs = [xf[n].astype(bf16) for n in range(N)]

    in_maps = []
    for c in range(NCORES):
        n, ch = c // 4, c % 4
        isl = slice(ch * CHUNK, (ch + 1) * CHUNK)
        in_maps.append({
            "xb": xbs[n],
            "wx": np.ascontiguousarray(
                np.concatenate([wall, xbs[n][:, isl]], axis=1)),
            "xres": np.ascontiguousarray(xf[n][:, isl]),
            "ball": ball, "brows": brows,
        })
    return in_maps


def kernel(x=None, wq=None, bq=None, wk=None, bk=None, wv=None, bv=None,
           wl=None, bl=None, gamma=None, num_heads=None, **_unused):
    from concourse import bass_utils

    x = np.asarray(x, dtype=np.float32)
    assert x.shape == (N, C, H, W), f"unexpected x shape {x.shape}"
    assert int(np.asarray(num_heads)) == NH

    in_maps = _prep_in_maps(x, wq, bq, wk, bk, wv, bv, wl, bl, gamma)
    nc = _get_nc()
    last_err = None
    for _attempt in range(3):
        try:
            res = bass_utils.run_bass_kernel_spmd(nc, in_maps,
                                                  core_ids=list(range(NCORES)))
            break
        except Exception as e:  # transient NRT device wedges have been seen
            last_err = e
            import time as _time
            _time.sleep(2.0)
    else:
        raise last_err

    out = np.empty((N, C, L), np.float32)
    for c in range(NCORES):
        n, ch = c // 4, c % 4
        out[n][:, ch * CHUNK:(ch + 1) * CHUNK] = res.results[c]["y"]
    return out.reshape(N, C, H, W)


if __name__ == "__main__":
    import reference
    inputs = reference.setup_inputs()
    expected = np.asarray(reference.reference(**inputs))
    got = kernel(**{k: np.asarray(v) if hasattr(v, "shape") else v
                    for k, v in inputs.items()})
    rel = np.linalg.norm(got - expected) / np.linalg.norm(expected)
    print("rel err:", rel)
